# revision 29
# baseline (speedup 1.0000x reference)
"""Trainium2 Bass kernel for nn_AudioPreprocessor (binaural STFT features).

Contract: kernel(**inputs) takes the FULL unsharded inputs (numpy) and
returns the full [8, 6, 64, 1001] float32 output. Internally: data-parallel
over batch across 8 NeuronCores (one batch per core, no collectives).

Pipeline per core (batch b):
  A) STFT of L/R channels as DFT-matmuls (f32r, weights pre-scaled by 1/16
     so pow/csd fit fp8e4 range), two time-halves (weights streamed twice,
     not four times); per (half, freq-tile): L channel accumulates in PSUM,
     drained to SBUF, then R channel; derive pow_L/R, csd_r/i into fp8e4
     residents (scaled 1/256, freq-padded 1025->1280 = 10 k-tiles for
     DoubleRow).  Mel matmuls run inline, deferred one freq-tile to keep
     the PE stream dense: csd path in true fp32 (ch3/ch4 are
     ill-conditioned), pow path in fp8.  No DRAM staging.
  B) per direction tile (18 x 128 dirs): n1/n2/corr_un as fp8 DoubleRow
     matmuls (2 k-tiles / 256 contraction per pass), normalize
     corr = corr_un * exp(-0.5*ln(n1*n2+eps)), scatter into azimuth bins
     via 0/1 matmul accumulated in PSUM.
  C) finalize ch0..ch4 right after phase A (overlaps phase B; all finalize
     work stays on the scalar queue so sync/gpsimd can start phase-B weight
     loads immediately); ch2 = ch0-ch1 algebraically; ch5 after the scatter.
"""
import sys
import os

_RL = "/opt/trn_rl_repo"
if _RL not in sys.path:
    sys.path.insert(0, _RL)

import numpy as np
import ml_dtypes
from contextlib import ExitStack

import concourse.bass as bass
import concourse.mybir as mybir
import concourse.tile as tile
from concourse.bass_utils import run_bass_kernel_spmd

# ---------------- problem constants (hardcoded) ----------------
B = 8
NS = 960000
NFFT = 2048
HOP = 960
F = 1025
FP = 1152            # padded freq dim: 9 * 128
NFT = FP // 128      # 9 freq tiles
NFTP = 10            # fp8 resid freq tiles (padded to even for DoubleRow)
NFM = 6              # freq tiles feeding mel (mel_fb zero above bin 683)
NCH = 16             # 2048 / 128 contraction chunks
NM = 64
ND = 2304
NDT = ND // 128      # 18 direction tiles
T = 1001
TC = 1002           # computed columns (f32r matmul needs even N); last col discarded
EPS = 1e-8
PADNS = NS + NFFT + HOP   # 963008 (one extra hop so we can compute T_C=1002 columns)
K_LOG = 10.0 / np.log(10.0)

# fp8 scaling: DFT weights pre-scaled by S_X -> X*S_X -> pow/csd * S_X^2.
S_X = 1.0 / 16.0
S_P = S_X * S_X                      # 1/256 scale on pow/csd
EPS_B = np.float32(EPS) * S_P * S_P  # phase-B rsqrt bias (n1*n2 scaled by S_P^2)
EPS_M = np.float32(EPS) * S_P        # mel ln bias (mel pow scaled by S_P)
C_LOG = 10.0 * np.log10(1.0 / S_P)   # ch0/ch1 offset compensating S_P

T_HALVES = [(0, 512), (512, 490)]    # matmul N<=512 splits of TC

F32 = mybir.dt.float32
F32R = mybir.dt.float32r
BF16 = mybir.dt.bfloat16
FP8 = mybir.dt.float8e4
AF = mybir.ActivationFunctionType
DROW = mybir.MatmulPerfMode.DoubleRow

_NC_CACHE = {}


def _budget(inst) -> int:
    return 1


def fix_sync_waits(nc):
    """Split per-instruction sync waits: this walrus build allows only ONE
    sync wait per instruction; hoist extras onto NoOps just before, on the
    same engine queue (engine streams execute in block order)."""
    counter = 0
    for f in nc.m.functions:
        for bb in f.blocks:
            new_insts = []
            changed = False
            for inst in bb.instructions:
                si = inst.sync_info
                waits = list(si.on_wait) if si is not None and si.on_wait else []
                budget = _budget(inst)
                if len(waits) > budget:
                    keep = waits[len(waits) - budget:]
                    excess = waits[: len(waits) - budget]
                    for wt in excess:
                        counter += 1
                        nop = mybir.InstNoOp(name=f"I-waitfix-{counter}")
                        nop.engine = inst.engine
                        nop.sync_info = mybir.SyncInfo(on_wait=[wt], on_update=[])
                        new_insts.append(nop)
                    inst.sync_info = mybir.SyncInfo(
                        on_wait=keep,
                        on_update=list(si.on_update) if si.on_update else [],
                    )
                    changed = True
                new_insts.append(inst)
            if changed:
                bb.instructions = new_insts
    return nc


def build_nc():
    nc = bass.Bass()

    frames = nc.declare_dram_parameter("frames", [2, 2, NCH, 128, 512], F32, isOutput=False)
    dftc = nc.declare_dram_parameter("dftc", [NFT, 128, NCH, 128], F32, isOutput=False)
    dfts = nc.declare_dram_parameter("dfts", [NFT, 128, NCH, 128], F32, isOutput=False)
    wwr = nc.declare_dram_parameter("wwr", [NDT, 128, NFTP, 128], FP8, isOutput=False)
    wwi = nc.declare_dram_parameter("wwi", [NDT, 128, NFTP, 128], FP8, isOutput=False)
    whr = nc.declare_dram_parameter("whr", [NDT, 128, NFTP, 128], FP8, isOutput=False)
    whl = nc.declare_dram_parameter("whl", [NDT, 128, NFTP, 128], FP8, isOutput=False)
    melw = nc.declare_dram_parameter("melw", [128, NFM, NM], FP8, isOutput=False)
    melwf = nc.declare_dram_parameter("melwf", [128, NFM, NM], F32, isOutput=False)
    sT2w = nc.declare_dram_parameter("sT2w", [NDT // 2, 128, 2, NM], FP8, isOutput=False)
    rcnt = nc.declare_dram_parameter("rcnt", [NM, 1], F32, isOutput=False)
    out = nc.declare_dram_parameter("out", [6, NM, T], F32, isOutput=True)

    with tile.TileContext(nc) as tc, ExitStack() as ctx:
        # resident pow/csd arrays [128, 10, 1002] fp8 (scaled by 1/256)
        resid = ctx.enter_context(tc.tile_pool(name="resid", bufs=1))
        powL = resid.tile([128, NFTP, TC], FP8, tag="powL")
        powR = resid.tile([128, NFTP, TC], FP8, tag="powR")
        csdR = resid.tile([128, NFTP, TC], FP8, tag="csdR")
        csdI = resid.tile([128, NFTP, TC], FP8, tag="csdI")
        # zero the padded k-tile (9): DoubleRow reads it; weights there are 0,
        # but uninitialized fp8 could be inf/nan and 0*inf = nan in the PE.
        for rt_ in (powL, powR, csdR, csdI):
            nc.vector.memset(rt_[:, 9, :], 0.0)
        # mel/channel tiles [64, 1002] f32 + consts
        mels = ctx.enter_context(tc.tile_pool(name="mels", bufs=1))
        melLR = mels.tile([128, TC], F32, tag="melLR")
        melri = mels.tile([128, TC], F32, tag="melri")
        melL_s, melR_s = melLR[0:NM, :], melLR[NM:128, :]
        melr_s, meli_s = melri[0:NM, :], melri[NM:128, :]
        ebB = mels.tile([128, 1], F32, tag="ebB")      # phase-B bias eps*S_P^2
        ebM = mels.tile([128, 1], F32, tag="ebM")      # mel ln bias eps*S_P
        ebN = mels.tile([NM, 1], F32, tag="ebN")       # ch3/4 norm bias eps
        cb0 = mels.tile([128, 1], F32, tag="cb0")      # ch0/1 +10*log10(256)
        rcnt_s = mels.tile([NM, 1], F32, tag="rcnt")
        nc.vector.memset(ebB, float(EPS_B))
        nc.vector.memset(ebM, float(EPS_M))
        nc.vector.memset(ebN, float(EPS))
        nc.vector.memset(cb0, float(C_LOG))
        # mel weights resident through phase A (tiles allocated here; their
        # DMAs are emitted after the half-0 frame loads so the prologue's
        # critical wc0+fL0 transfers aren't queued behind them)
        melwp = ctx.enter_context(tc.tile_pool(name="melwp", bufs=1))
        melw_s = melwp.tile([128, NFM, NM], FP8, tag="melw")
        melwf_s = melwp.tile([128, NFM, NM], F32, tag="melwf")
        # phase-B weight pools live for the whole kernel so the first two
        # direction tiles can prefetch during phase A's second half
        wBp = ctx.enter_context(tc.tile_pool(name="wB", bufs=4))
        sTp = ctx.enter_context(tc.tile_pool(name="sTp", bufs=2))
        prefetchB = {}

        # ---------------- Phase A: STFT + pow/csd + inline mel -------------
        with tc.tile_pool(name="frames", bufs=1) as fpool, \
             tc.tile_pool(name="dftw", bufs=2) as wpool, \
             tc.tile_pool(name="pa2", bufs=2) as tp2, \
             tc.tile_pool(name="pa1", bufs=1) as tp1, \
             tc.tile_pool(name="psL", bufs=1, space="PSUM") as psL, \
             tc.tile_pool(name="psR", bufs=2, space="PSUM") as psR, \
             tc.tile_pool(name="psmel", bufs=1, space="PSUM") as psMel:
            for hx, (t0, hc) in enumerate(T_HALVES):
                hsl = slice(t0, t0 + hc)
                fL, fR = {}, {}
                if hx == 0:
                    # fi-0 weights first: the very first matmul only needs
                    # wc0 + fL[0], so the PE starts ~7us in instead of ~33us
                    wc0 = wpool.tile([128, NCH, 128], F32R, tag="wc")
                    nc.sync.dma_start(out=wc0, in_=dftc[0].bitcast(F32R))
                    ws0 = wpool.tile([128, NCH, 128], F32R, tag="ws")
                    nc.gpsimd.dma_start(out=ws0, in_=dfts[0].bitcast(F32R))
                for c in range(NCH):
                    ft = fpool.tile([128, 512], F32R, tag=f"fL{c}")
                    eng = nc.sync if (c % 2 == 0) else nc.gpsimd
                    eng.dma_start(out=ft, in_=frames[0, hx, c].bitcast(F32R))
                    fL[c] = ft
                if hx == 1:
                    wc0 = wpool.tile([128, NCH, 128], F32R, tag="wc")
                    nc.sync.dma_start(out=wc0, in_=dftc[0].bitcast(F32R))
                    ws0 = wpool.tile([128, NCH, 128], F32R, tag="ws")
                    nc.gpsimd.dma_start(out=ws0, in_=dfts[0].bitcast(F32R))
                for c in range(NCH):
                    ft = fpool.tile([128, 512], F32R, tag=f"fR{c}")
                    eng = nc.sync if (c % 2 == 0) else nc.gpsimd
                    eng.dma_start(out=ft, in_=frames[1, hx, c].bitcast(F32R))
                    fR[c] = ft
                if hx == 0:
                    nc.sync.dma_start(out=rcnt_s, in_=rcnt[:, :])
                    nc.sync.dma_start(out=melw_s, in_=melw[:, :, :])
                    nc.gpsimd.dma_start(out=melwf_s, in_=melwf[:, :, :])
                if hx == 1:
                    # prefetch the first two phase-B direction tiles' weights
                    # during the second STFT half, so phase B starts hot
                    for dj in (0, 1):
                        pw = {}
                        pw["whr"] = wBp.tile([128, NFTP, 128], FP8, tag="whr", name="pwhr")
                        nc.sync.dma_start(out=pw["whr"], in_=whr[dj])
                        pw["whl"] = wBp.tile([128, NFTP, 128], FP8, tag="whl", name="pwhl")
                        nc.gpsimd.dma_start(out=pw["whl"], in_=whl[dj])
                        pw["wwr"] = wBp.tile([128, NFTP, 128], FP8, tag="wwr", name="pwwr")
                        nc.sync.dma_start(out=pw["wwr"], in_=wwr[dj])
                        pw["wwi"] = wBp.tile([128, NFTP, 128], FP8, tag="wwi", name="pwwi")
                        nc.gpsimd.dma_start(out=pw["wwi"], in_=wwi[dj])
                        prefetchB[dj] = pw
                    sT2p = sTp.tile([128, 2, NM], FP8, tag="sT2")
                    nc.sync.dma_start(out=sT2p, in_=sT2w[0])
                    prefetchB["sT2_0"] = sT2p
                pend_mel = []
                for fi in range(NFT):
                    if fi == 0:
                        wc, ws = wc0, ws0
                    else:
                        wc = wpool.tile([128, NCH, 128], F32R, tag="wc")
                        nc.sync.dma_start(out=wc, in_=dftc[fi].bitcast(F32R))
                        ws = wpool.tile([128, NCH, 128], F32R, tag="ws")
                        nc.gpsimd.dma_start(out=ws, in_=dfts[fi].bitcast(F32R))
                    cL = psL.tile([128, hc], F32, tag="cL")
                    sL = psL.tile([128, hc], F32, tag="sL")
                    for c in range(NCH):
                        st, sp = (c == 0), (c == NCH - 1)
                        nc.tensor.matmul(cL, wc[:, c, :], fL[c][:, :hc],
                                         start=st, stop=sp, skip_group_check=True)
                        nc.tensor.matmul(sL, ws[:, c, :], fL[c][:, :hc],
                                         start=st, stop=sp, skip_group_check=True)
                    cR = psR.tile([128, hc], F32, tag="cR")
                    sR = psR.tile([128, hc], F32, tag="sR")
                    for c in range(NCH):
                        st, sp = (c == 0), (c == NCH - 1)
                        nc.tensor.matmul(cR, wc[:, c, :], fR[c][:, :hc],
                                         start=st, stop=sp, skip_group_check=True)
                        nc.tensor.matmul(sR, ws[:, c, :], fR[c][:, :hc],
                                         start=st, stop=sp, skip_group_check=True)
                    # emit the previous fi's mel matmuls now, so the PE never
                    # stalls on this fi's DVE/Act derivation chain
                    for mm_args in pend_mel:
                        nc.tensor.matmul(*mm_args[0], **mm_args[1])
                    pend_mel = []
                    if fi == NFM:
                        # fi-5's mel matmuls (the group stop) were just
                        # emitted: drain the mel accumulators now, well
                        # before the end-of-half scalar backlog
                        nc.scalar.copy(melri[:, hsl], pmc)
                        nc.scalar.copy(melLR[:, hsl], pmp)
                    # drain L PSUM early (psL bufs=1 gates the next fi)
                    aLr = tp2.tile([128, hc], F32, tag="aLr")
                    nc.scalar.copy(aLr, cL)
                    aLi = tp2.tile([128, hc], F32, tag="aLi")
                    nc.scalar.copy(aLi, sL)
                    # pow_L from SBUF copies; pow_R straight from PSUM
                    p1 = tp2.tile([128, hc], F32, tag="p1")
                    nc.scalar.activation(p1, aLr, AF.Square)
                    p2 = tp2.tile([128, hc], F32, tag="p2")
                    nc.scalar.activation(p2, aLi, AF.Square)
                    nc.vector.tensor_add(powL[:, fi, hsl], p1, p2)
                    p3 = tp2.tile([128, hc], F32, tag="p3")
                    nc.scalar.activation(p3, cR, AF.Square)
                    p4 = tp2.tile([128, hc], F32, tag="p4")
                    nc.scalar.activation(p4, sR, AF.Square)
                    nc.vector.tensor_add(powR[:, fi, hsl], p3, p4)
                    # csd = conj(X_L)*X_R ... (X_L * conj(X_R)): r = LrRr+LiRi,
                    # i = LiRr-LrRi
                    m1 = tp1.tile([128, hc], F32, tag="m1")
                    nc.vector.tensor_mul(m1, aLr, cR)
                    m2 = tp1.tile([128, hc], F32, tag="m2")
                    nc.vector.tensor_mul(m2, aLi, sR)
                    csdf = tp2.tile([128, hc], F32, tag="csdf")
                    nc.vector.tensor_add(csdf, m1, m2)
                    nc.scalar.copy(csdR[:, fi, hsl], csdf)
                    m3 = tp1.tile([128, hc], F32, tag="m3")
                    nc.vector.tensor_mul(m3, aLi, cR)
                    m4 = tp1.tile([128, hc], F32, tag="m4")
                    nc.vector.tensor_mul(m4, aLr, sR)
                    csif = tp2.tile([128, hc], F32, tag="csif")
                    nc.vector.tensor_sub(csif, m3, m4)
                    nc.scalar.copy(csdI[:, fi, hsl], csif)
                    if fi == 0:
                        pmc = psMel.tile([128, hc], F32, tag="pmc")
                        pmp = psMel.tile([128, hc], F32, tag="pmp")
                    if fi < NFM:
                        st, sp = (fi == 0), (fi == NFM - 1)
                        # csd mel in true fp32 (4 cyc/row): ch3/ch4 accuracy
                        pend_mel.append((
                            (pmc[0:NM, :], melwf_s[:, fi, :], csdf),
                            dict(start=st, stop=sp, skip_group_check=True)))
                        pend_mel.append((
                            (pmc[NM:128, :], melwf_s[:, fi, :], csif),
                            dict(start=st, stop=sp, skip_group_check=True)))
                        # pow mel in fp8
                        pend_mel.append((
                            (pmp[0:NM, :], melw_s[:, fi, :], powL[:, fi, hsl]),
                            dict(start=st, stop=sp, skip_group_check=True)))
                        pend_mel.append((
                            (pmp[NM:128, :], melw_s[:, fi, :], powR[:, fi, hsl]),
                            dict(start=st, stop=sp, skip_group_check=True)))
                for mm_args in pend_mel:
                    nc.tensor.matmul(*mm_args[0], **mm_args[1])
                if hx == 1:
                    for dj in (2, 3):
                        pw = {}
                        pw["whr"] = wBp.tile([128, NFTP, 128], FP8, tag="whr", name="pwhr2")
                        nc.sync.dma_start(out=pw["whr"], in_=whr[dj])
                        pw["whl"] = wBp.tile([128, NFTP, 128], FP8, tag="whl", name="pwhl2")
                        nc.gpsimd.dma_start(out=pw["whl"], in_=whl[dj])
                        pw["wwr"] = wBp.tile([128, NFTP, 128], FP8, tag="wwr", name="pwwr2")
                        nc.sync.dma_start(out=pw["wwr"], in_=wwr[dj])
                        pw["wwi"] = wBp.tile([128, NFTP, 128], FP8, tag="wwi", name="pwwi2")
                        nc.gpsimd.dma_start(out=pw["wwi"], in_=wwi[dj])
                        prefetchB[dj] = pw
                    sT2q = sTp.tile([128, 2, NM], FP8, tag="sT2")
                    nc.sync.dma_start(out=sT2q, in_=sT2w[1])
                    prefetchB["sT2_1"] = sT2q

        # ---------------- Phase C (ch0-ch4): finalize early ----------------
        # Only depends on phase A; output DMAs go on the vector queue so the
        # sync/gpsimd queues can start phase-B weight loads immediately.
        with tc.tile_pool(name="fint", bufs=1) as fin:
            # ch0/ch1 fused on the packed [128, TC] tile:
            # 10*log10(mel+eps) = K*ln(mel_s + eps*S_P) + C_LOG
            u01 = fin.tile([128, TC], F32, tag="u01")
            nc.scalar.activation(u01, melLR, AF.Ln, bias=ebM)
            c01 = fin.tile([128, TC], F32, tag="c01")
            nc.scalar.activation(c01, u01, AF.Identity, bias=cb0, scale=K_LOG)
            nc.scalar.dma_start(out=out[0], in_=c01[0:NM, :T])
            nc.scalar.dma_start(out=out[1], in_=c01[NM:128, :T])
            # ch2 = ch0 - ch1 up to O(eps/mel) terms; base-shift u1 to
            # partitions 0-63 first (DVE needs equal base partitions)
            u1c = fin.tile([NM, TC], F32, tag="u1c")
            nc.scalar.copy(u1c, u01[NM:128, :])
            w2 = fin.tile([NM, TC], F32, tag="w2")
            nc.vector.tensor_sub(w2, u01[0:NM, :], u1c)
            c2 = fin.tile([NM, TC], F32, tag="c2")
            nc.scalar.mul(c2, w2, K_LOG)
            nc.scalar.dma_start(out=out[2], in_=c2[:, :T])
            # ch3/ch4: rn = exp(-0.5*ln(r^2+i^2+eps))
            sq = fin.tile([128, TC], F32, tag="sq")
            nc.scalar.activation(sq, melri, AF.Square)
            sqi = fin.tile([NM, TC], F32, tag="sqi")
            nc.scalar.copy(sqi, sq[NM:128, :])
            ss = fin.tile([NM, TC], F32, tag="ss")
            nc.vector.tensor_add(ss, sq[0:NM, :], sqi)
            lnss = fin.tile([NM, TC], F32, tag="lnss")
            nc.scalar.activation(lnss, ss, AF.Ln, bias=ebN)
            rn = fin.tile([NM, TC], F32, tag="rn")
            nc.scalar.activation(rn, lnss, AF.Exp, scale=-0.5)
            mic = fin.tile([NM, TC], F32, tag="mic")
            nc.scalar.copy(mic, melri[NM:128, :])
            c3 = fin.tile([NM, TC], F32, tag="c3")
            nc.vector.tensor_mul(c3, mic, rn)
            nc.scalar.dma_start(out=out[3], in_=c3[:, :T])
            c4 = fin.tile([NM, TC], F32, tag="c4")
            nc.vector.tensor_mul(c4, melri[0:NM, :], rn)
            nc.scalar.dma_start(out=out[4], in_=c4[:, :T])

        # ---------------- Phase B: corr + scatter (fp8 DoubleRow) ----------
        # halves-inner ordering: consecutive matmul pairs share the same
        # lhsT slice, halving distinct weight loads if codegen elides them.
        with tc.tile_pool(name="pbt", bufs=2) as pbt, \
             tc.tile_pool(name="corrp", bufs=2) as corrp, \
             tc.tile_pool(name="psumB", bufs=1, space="PSUM") as psB, \
             tc.tile_pool(name="psumB5", bufs=1, space="PSUM") as psB5:
            ch5p = psB5.tile([NM, TC], F32, tag="ch5")
            pend_scatter = []
            c2 = None
            sT2 = None
            for dj in range(NDT):
                if dj in prefetchB:
                    pw = prefetchB.pop(dj)
                    whr_t, whl_t = pw["whr"], pw["whl"]
                    wwr_t, wwi_t = pw["wwr"], pw["wwi"]
                else:
                    whr_t = wBp.tile([128, NFTP, 128], FP8, tag="whr")
                    nc.sync.dma_start(out=whr_t, in_=whr[dj])
                    whl_t = wBp.tile([128, NFTP, 128], FP8, tag="whl")
                    nc.gpsimd.dma_start(out=whl_t, in_=whl[dj])
                    wwr_t = wBp.tile([128, NFTP, 128], FP8, tag="wwr")
                    nc.sync.dma_start(out=wwr_t, in_=wwr[dj])
                    wwi_t = wBp.tile([128, NFTP, 128], FP8, tag="wwi")
                    nc.gpsimd.dma_start(out=wwi_t, in_=wwi[dj])
                if dj % 2 == 0:
                    key = f"sT2_{dj // 2}"
                    if key in prefetchB:
                        sT2 = prefetchB.pop(key)
                    else:
                        sT2 = sTp.tile([128, 2, NM], FP8, tag="sT2")
                        nc.sync.dma_start(out=sT2, in_=sT2w[dj // 2])
                    c2 = [corrp.tile([128, 2, hc], FP8, tag=f"c2_{hh}",
                                     name=f"c2_{hh}")
                          for hh, (h0, hc) in enumerate(T_HALVES)]

                n1 = [psB.tile([128, hc], F32, tag=f"n1_{hh}", name=f"n1_{hh}")
                      for hh, (h0, hc) in enumerate(T_HALVES)]
                n2 = [psB.tile([128, hc], F32, tag=f"n2_{hh}", name=f"n2_{hh}")
                      for hh, (h0, hc) in enumerate(T_HALVES)]
                cu = [psB.tile([128, hc], F32, tag=f"cu_{hh}", name=f"cu_{hh}")
                      for hh, (h0, hc) in enumerate(T_HALVES)]
                # n1/n2 first, cu last: at the dj boundary the previous
                # dj's cu accumulator is freed by the END of its derivation
                # chain, so the next dj must not need cu's PSUM immediately
                for k in range(NFTP // 2):
                    ksl = slice(2 * k, 2 * k + 2)
                    st, sp = (k == 0), (k == NFTP // 2 - 1)
                    for hh, (h0, hc) in enumerate(T_HALVES):
                        nc.tensor.matmul(n1[hh], whr_t[:, ksl, :],
                                         powL[:, ksl, h0:h0 + hc],
                                         start=st, stop=sp,
                                         perf_mode=DROW, skip_group_check=True)
                    for hh, (h0, hc) in enumerate(T_HALVES):
                        nc.tensor.matmul(n2[hh], whl_t[:, ksl, :],
                                         powR[:, ksl, h0:h0 + hc],
                                         start=st, stop=sp,
                                         perf_mode=DROW, skip_group_check=True)
                for k in range(NFTP // 2):
                    ksl = slice(2 * k, 2 * k + 2)
                    st, sp = (k == 0), (k == NFTP // 2 - 1)
                    for hh, (h0, hc) in enumerate(T_HALVES):
                        nc.tensor.matmul(cu[hh], wwr_t[:, ksl, :],
                                         csdR[:, ksl, h0:h0 + hc],
                                         start=st, stop=False,
                                         perf_mode=DROW, skip_group_check=True)
                    for hh, (h0, hc) in enumerate(T_HALVES):
                        nc.tensor.matmul(cu[hh], wwi_t[:, ksl, :],
                                         csdI[:, ksl, h0:h0 + hc],
                                         start=False, stop=sp,
                                         perf_mode=DROW, skip_group_check=True)
                den = pbt.tile([128, TC], F32, tag="den")
                for hh, (h0, hc) in enumerate(T_HALVES):
                    cn1 = pbt.tile([128, hc], F32, tag="cn1")
                    nc.scalar.copy(cn1, n1[hh])
                    nc.vector.tensor_mul(den[:, h0:h0 + hc], cn1, n2[hh])
                # one Ln/Exp over the full row instead of two per-half ops:
                # halves the scalar-queue op count per direction tile
                lnd = pbt.tile([128, TC], F32, tag="lnd")
                nc.scalar.activation(lnd, den, AF.Ln, bias=ebB)
                rden = pbt.tile([128, TC], F32, tag="rden")
                nc.scalar.activation(rden, lnd, AF.Exp, scale=-0.5)
                for hh, (h0, hc) in enumerate(T_HALVES):
                    nc.vector.tensor_mul(c2[hh][:, dj % 2, :], cu[hh],
                                         rden[:, h0:h0 + hc])
                if dj % 2 == 1:
                    # defer the pair's scatter until after the NEXT pair's
                    # matmuls so the PE never stalls on the DVE chain
                    pend_scatter.append((sT2, c2, dj // 2))
                    if len(pend_scatter) > 1:
                        psT2, pc2, pp = pend_scatter.pop(0)
                        for hh, (h0, hc) in enumerate(T_HALVES):
                            nc.tensor.matmul(ch5p[:, h0:h0 + hc], psT2, pc2[hh],
                                             start=(pp == 0), stop=False,
                                             perf_mode=DROW,
                                             skip_group_check=True)
            for kk, (psT2, pc2, pp) in enumerate(pend_scatter):
                last = (kk == len(pend_scatter) - 1)
                for hh, (h0, hc) in enumerate(T_HALVES):
                    nc.tensor.matmul(ch5p[:, h0:h0 + hc], psT2, pc2[hh],
                                     start=(pp == 0), stop=last,
                                     perf_mode=DROW, skip_group_check=True)

            # ---------------- ch5 epilogue ----------------
            with tc.tile_pool(name="fin5", bufs=1) as fin5:
                c5 = fin5.tile([NM, TC], F32, tag="c5")
                nc.vector.tensor_scalar_mul(c5, ch5p, rcnt_s)
                nc.sync.dma_start(out=out[5], in_=c5[:, :T])

    fix_sync_waits(nc)
    return nc


def _host_prep(inputs):
    wav = np.asarray(inputs["waveform"], dtype=np.float32)          # [8,2,NS]
    W_real = np.asarray(inputs["W_real"], dtype=np.float32)         # [ND,F]
    W_imag = np.asarray(inputs["W_imag"], dtype=np.float32)
    norm_hr = np.asarray(inputs["norm_hr_sq"], dtype=np.float32)
    norm_hl = np.asarray(inputs["norm_hl_sq"], dtype=np.float32)
    az = np.asarray(inputs["az_bin_idx"]).astype(np.int64)          # [ND]
    win = np.asarray(inputs["window"], dtype=np.float32)            # [NFFT]
    mel_fb = np.asarray(inputs["mel_fb"], dtype=np.float32)         # [NM,F]

    xpad = np.pad(wav, ((0, 0), (0, 0), (NFFT // 2, NFFT // 2 + HOP)))  # [8,2,PADNS]
    # host framing: each (half, channel, chunk) SBUF tile is one fully
    # contiguous 256KB block in DRAM (single-descriptor DMAs)
    sw = np.lib.stride_tricks.sliding_window_view(xpad, NFFT, axis=2)[:, :, ::HOP]
    # sw: [8, 2, TC, NFFT] view -> [8, 2, NFFT, TC]
    frames_np = np.ascontiguousarray(np.swapaxes(sw, 2, 3))
    fr = np.zeros((B, 2, 2, NCH, 128, 512), dtype=np.float32)
    for hx, (t0, hc) in enumerate(T_HALVES):
        fr[:, :, hx, :, :, :hc] = frames_np[:, :, :, t0:t0 + hc].reshape(
            B, 2, NCH, 128, hc)

    n = np.arange(NFFT, dtype=np.float64)[:, None]
    k = np.arange(F, dtype=np.float64)[None, :]
    ang = 2.0 * np.pi * n * k / NFFT
    dftc = np.zeros((NFFT, FP), dtype=np.float32)
    dfts = np.zeros((NFFT, FP), dtype=np.float32)
    dftc[:, :F] = (np.cos(ang) * win[:, None] * S_X).astype(np.float32)
    dfts[:, :F] = (-np.sin(ang) * win[:, None] * S_X).astype(np.float32)

    def tile_dft(a):  # [NFFT, FP] -> [NFT, 128, NCH, 128]
        return np.ascontiguousarray(
            a.reshape(NCH, 128, NFT, 128).transpose(2, 1, 0, 3))

    def tile_w(mat):  # [ND, F] -> lhsT tiled [NDT, 128, NFTP, 128] fp8e4
        t = np.zeros((NFTP * 128, ND), dtype=np.float32)
        t[:F] = mat.T
        return np.ascontiguousarray(
            t.reshape(NFTP, 128, NDT, 128).transpose(2, 1, 0, 3)
        ).astype(ml_dtypes.float8_e4m3)

    wwr = tile_w(W_real)
    wwi = tile_w(-W_imag)
    whr = tile_w(norm_hr)
    whl = tile_w(norm_hl)

    # mel_fb is zero for bins >= 684, so dropping freq tiles 6-8 is exact
    melT = np.array(mel_fb.T[:NFM * 128], dtype=np.float32)  # [768, NM]
    mel_base = melT.reshape(NFM, 128, NM).transpose(1, 0, 2)  # [128, NFM, NM]
    melwf = np.ascontiguousarray(mel_base * np.float32(1.0 / S_P), dtype=np.float32)
    melw = np.ascontiguousarray(mel_base).astype(ml_dtypes.float8_e4m3)

    cnt = np.bincount(az, minlength=NM).astype(np.float32)
    S01 = (az[:, None] == np.arange(NM)[None, :]).astype(np.float32)  # [ND,NM]
    sT2w = np.ascontiguousarray(
        S01.reshape(NDT // 2, 2, 128, NM).transpose(0, 2, 1, 3)
    ).astype(ml_dtypes.float8_e4m3)
    rcnt = (1.0 / (cnt + EPS)).astype(np.float32).reshape(NM, 1)

    shared = {
        "dftc": tile_dft(dftc), "dfts": tile_dft(dfts),
        "wwr": wwr, "wwi": wwi, "whr": whr, "whl": whl,
        "melw": melw, "melwf": melwf, "sT2w": sT2w, "rcnt": rcnt,
    }
    in_maps = []
    for b in range(B):
        m = dict(shared)
        m["frames"] = fr[b]
        in_maps.append(m)
    return in_maps


def kernel(**inputs) -> np.ndarray:
    if "nc" not in _NC_CACHE:
        _NC_CACHE["nc"] = build_nc()
    nc = _NC_CACHE["nc"]
    in_maps = _host_prep(inputs)
    res = run_bass_kernel_spmd(nc, in_maps, core_ids=list(range(B)))
    out = np.stack([np.asarray(res.results[i]["out"]) for i in range(B)])
    return out.astype(np.float32)


# revision 31
# speedup vs baseline: 1.0132x; 1.0132x over previous
"""Trainium2 Bass kernel for nn_AudioPreprocessor (binaural STFT features).

Contract: kernel(**inputs) takes the FULL unsharded inputs (numpy) and
returns the full [8, 6, 64, 1001] float32 output. Internally: data-parallel
over batch across 8 NeuronCores (one batch per core, no collectives).

Pipeline per core (batch b):
  A) STFT of L/R channels as DFT-matmuls (f32r, weights pre-scaled by 1/16
     so pow/csd fit fp8e4 range), two time-halves (weights streamed twice,
     not four times); per (half, freq-tile): L channel accumulates in PSUM,
     drained to SBUF, then R channel; derive pow_L/R, csd_r/i into fp8e4
     residents (scaled 1/256, freq-padded 1025->1280 = 10 k-tiles for
     DoubleRow).  Mel matmuls run inline, deferred one freq-tile to keep
     the PE stream dense: csd path in true fp32 (ch3/ch4 are
     ill-conditioned), pow path in fp8.  No DRAM staging.
  B) per direction tile (18 x 128 dirs): n1/n2/corr_un as fp8 DoubleRow
     matmuls (2 k-tiles / 256 contraction per pass), normalize
     corr = corr_un * exp(-0.5*ln(n1*n2+eps)), scatter into azimuth bins
     via 0/1 matmul accumulated in PSUM.
  C) finalize ch0..ch4 right after phase A (overlaps phase B; all finalize
     work stays on the scalar queue so sync/gpsimd can start phase-B weight
     loads immediately); ch2 = ch0-ch1 algebraically; ch5 after the scatter.
"""
import sys
import os

_RL = "/opt/trn_rl_repo"
if _RL not in sys.path:
    sys.path.insert(0, _RL)

import numpy as np
import ml_dtypes
from contextlib import ExitStack

import concourse.bass as bass
import concourse.mybir as mybir
import concourse.tile as tile
from concourse.bass_utils import run_bass_kernel_spmd

# ---------------- problem constants (hardcoded) ----------------
B = 8
NS = 960000
NFFT = 2048
HOP = 960
F = 1025
FP = 1152            # padded freq dim: 9 * 128
NFT = FP // 128      # 9 freq tiles
NFTP = 10            # fp8 resid freq tiles (padded to even for DoubleRow)
NFM = 6              # freq tiles feeding mel (mel_fb zero above bin 683)
NCH = 16             # 2048 / 128 contraction chunks
NM = 64
ND = 2304
NDT = ND // 128      # 18 direction tiles
T = 1001
TC = 1002           # computed columns (f32r matmul needs even N); last col discarded
EPS = 1e-8
PADNS = NS + NFFT + HOP   # 963008 (one extra hop so we can compute T_C=1002 columns)
K_LOG = 10.0 / np.log(10.0)

# fp8 scaling: DFT weights pre-scaled by S_X -> X*S_X -> pow/csd * S_X^2.
S_X = 1.0 / 16.0
S_P = S_X * S_X                      # 1/256 scale on pow/csd
EPS_B = np.float32(EPS) * S_P * S_P  # phase-B rsqrt bias (n1*n2 scaled by S_P^2)
EPS_M = np.float32(EPS) * S_P        # mel ln bias (mel pow scaled by S_P)
C_LOG = 10.0 * np.log10(1.0 / S_P)   # ch0/ch1 offset compensating S_P

T_HALVES = [(0, 512), (512, 490)]    # matmul N<=512 splits of TC

F32 = mybir.dt.float32
F32R = mybir.dt.float32r
BF16 = mybir.dt.bfloat16
FP8 = mybir.dt.float8e4
AF = mybir.ActivationFunctionType
DROW = mybir.MatmulPerfMode.DoubleRow

_NC_CACHE = {}


def _budget(inst) -> int:
    return 1


def fix_sync_waits(nc):
    """Split per-instruction sync waits: this walrus build allows only ONE
    sync wait per instruction; hoist extras onto NoOps just before, on the
    same engine queue (engine streams execute in block order)."""
    counter = 0
    for f in nc.m.functions:
        for bb in f.blocks:
            new_insts = []
            changed = False
            for inst in bb.instructions:
                si = inst.sync_info
                waits = list(si.on_wait) if si is not None and si.on_wait else []
                budget = _budget(inst)
                if len(waits) > budget:
                    keep = waits[len(waits) - budget:]
                    excess = waits[: len(waits) - budget]
                    for wt in excess:
                        counter += 1
                        nop = mybir.InstNoOp(name=f"I-waitfix-{counter}")
                        nop.engine = inst.engine
                        nop.sync_info = mybir.SyncInfo(on_wait=[wt], on_update=[])
                        new_insts.append(nop)
                    inst.sync_info = mybir.SyncInfo(
                        on_wait=keep,
                        on_update=list(si.on_update) if si.on_update else [],
                    )
                    changed = True
                new_insts.append(inst)
            if changed:
                bb.instructions = new_insts
    return nc


def build_nc():
    nc = bass.Bass()

    frames = nc.declare_dram_parameter("frames", [2, 2, NCH, 128, 512], F32, isOutput=False)
    dftc = nc.declare_dram_parameter("dftc", [NFT, 128, NCH, 128], F32, isOutput=False)
    dfts = nc.declare_dram_parameter("dfts", [NFT, 128, NCH, 128], F32, isOutput=False)
    wwr = nc.declare_dram_parameter("wwr", [NDT, 128, NFTP, 128], FP8, isOutput=False)
    wwi = nc.declare_dram_parameter("wwi", [NDT, 128, NFTP, 128], FP8, isOutput=False)
    whr = nc.declare_dram_parameter("whr", [NDT, 128, NFTP, 128], FP8, isOutput=False)
    whl = nc.declare_dram_parameter("whl", [NDT, 128, NFTP, 128], FP8, isOutput=False)
    melw = nc.declare_dram_parameter("melw", [128, NFM, NM], FP8, isOutput=False)
    melwf = nc.declare_dram_parameter("melwf", [128, NFM, NM], F32, isOutput=False)
    sT2w = nc.declare_dram_parameter("sT2w", [NDT // 2, 128, 2, NM], FP8, isOutput=False)
    rcnt = nc.declare_dram_parameter("rcnt", [NM, 1], F32, isOutput=False)
    out = nc.declare_dram_parameter("out", [6, NM, T], F32, isOutput=True)

    with tile.TileContext(nc) as tc, ExitStack() as ctx:
        # resident pow/csd arrays [128, 10, 1002] fp8 (scaled by 1/256)
        resid = ctx.enter_context(tc.tile_pool(name="resid", bufs=1))
        powL = resid.tile([128, NFTP, TC], FP8, tag="powL")
        powR = resid.tile([128, NFTP, TC], FP8, tag="powR")
        csdR = resid.tile([128, NFTP, TC], FP8, tag="csdR")
        csdI = resid.tile([128, NFTP, TC], FP8, tag="csdI")
        # zero the padded k-tile (9): DoubleRow reads it; weights there are 0,
        # but uninitialized fp8 could be inf/nan and 0*inf = nan in the PE.
        for rt_ in (powL, powR, csdR, csdI):
            nc.vector.memset(rt_[:, 9, :], 0.0)
        # mel/channel tiles [64, 1002] f32 + consts
        mels = ctx.enter_context(tc.tile_pool(name="mels", bufs=1))
        melLR = mels.tile([128, TC], F32, tag="melLR")
        melri = mels.tile([128, TC], F32, tag="melri")
        melL_s, melR_s = melLR[0:NM, :], melLR[NM:128, :]
        melr_s, meli_s = melri[0:NM, :], melri[NM:128, :]
        ebB = mels.tile([128, 1], F32, tag="ebB")      # phase-B bias eps*S_P^2
        ebM = mels.tile([128, 1], F32, tag="ebM")      # mel ln bias eps*S_P
        ebN = mels.tile([NM, 1], F32, tag="ebN")       # ch3/4 norm bias eps
        cb0 = mels.tile([128, 1], F32, tag="cb0")      # ch0/1 +10*log10(256)
        rcnt_s = mels.tile([NM, 1], F32, tag="rcnt")
        nc.vector.memset(ebB, float(EPS_B))
        nc.vector.memset(ebM, float(EPS_M))
        nc.vector.memset(ebN, float(EPS))
        nc.vector.memset(cb0, float(C_LOG))
        # mel weights resident through phase A (tiles allocated here; their
        # DMAs are emitted after the half-0 frame loads so the prologue's
        # critical wc0+fL0 transfers aren't queued behind them)
        melwp = ctx.enter_context(tc.tile_pool(name="melwp", bufs=1))
        melw_s = melwp.tile([128, NFM, NM], FP8, tag="melw")
        melwf_s = melwp.tile([128, NFM, NM], F32, tag="melwf")
        # phase-B weight pools live for the whole kernel so the first two
        # direction tiles can prefetch during phase A's second half
        wBp = ctx.enter_context(tc.tile_pool(name="wB", bufs=4))
        sTp = ctx.enter_context(tc.tile_pool(name="sTp", bufs=2))
        prefetchB = {}

        # ---------------- Phase A: STFT + pow/csd + inline mel -------------
        with tc.tile_pool(name="frames", bufs=1) as fpool, \
             tc.tile_pool(name="dftw", bufs=2) as wpool, \
             tc.tile_pool(name="pa2", bufs=2) as tp2, \
             tc.tile_pool(name="pa1", bufs=1) as tp1, \
             tc.tile_pool(name="psL", bufs=1, space="PSUM") as psL, \
             tc.tile_pool(name="psR", bufs=2, space="PSUM") as psR, \
             tc.tile_pool(name="psmel", bufs=1, space="PSUM") as psMel:
            for hx, (t0, hc) in enumerate(T_HALVES):
                hsl = slice(t0, t0 + hc)
                fL, fR = {}, {}
                if hx == 0:
                    # fi-0 weights first: the very first matmul only needs
                    # wc0 + fL[0], so the PE starts ~7us in instead of ~33us
                    wc0 = wpool.tile([128, NCH, 128], F32R, tag="wc")
                    nc.sync.dma_start(out=wc0, in_=dftc[0].bitcast(F32R))
                    ws0 = wpool.tile([128, NCH, 128], F32R, tag="ws")
                    nc.gpsimd.dma_start(out=ws0, in_=dfts[0].bitcast(F32R))
                for c in range(NCH):
                    ft = fpool.tile([128, 512], F32R, tag=f"fL{c}")
                    eng = nc.sync if (c % 2 == 0) else nc.gpsimd
                    eng.dma_start(out=ft, in_=frames[0, hx, c].bitcast(F32R))
                    fL[c] = ft
                if hx == 1:
                    wc0 = wpool.tile([128, NCH, 128], F32R, tag="wc")
                    nc.sync.dma_start(out=wc0, in_=dftc[0].bitcast(F32R))
                    ws0 = wpool.tile([128, NCH, 128], F32R, tag="ws")
                    nc.gpsimd.dma_start(out=ws0, in_=dfts[0].bitcast(F32R))
                for c in range(NCH):
                    ft = fpool.tile([128, 512], F32R, tag=f"fR{c}")
                    eng = nc.sync if (c % 2 == 0) else nc.gpsimd
                    eng.dma_start(out=ft, in_=frames[1, hx, c].bitcast(F32R))
                    fR[c] = ft
                if hx == 0:
                    nc.sync.dma_start(out=rcnt_s, in_=rcnt[:, :])
                    nc.sync.dma_start(out=melw_s, in_=melw[:, :, :])
                    nc.gpsimd.dma_start(out=melwf_s, in_=melwf[:, :, :])
                if hx == 1:
                    # prefetch the first two phase-B direction tiles' weights
                    # during the second STFT half, so phase B starts hot
                    for dj in (0, 1):
                        pw = {}
                        pw["whr"] = wBp.tile([128, NFTP, 128], FP8, tag="whr", name="pwhr")
                        nc.sync.dma_start(out=pw["whr"], in_=whr[dj])
                        pw["whl"] = wBp.tile([128, NFTP, 128], FP8, tag="whl", name="pwhl")
                        nc.gpsimd.dma_start(out=pw["whl"], in_=whl[dj])
                        pw["wwr"] = wBp.tile([128, NFTP, 128], FP8, tag="wwr", name="pwwr")
                        nc.sync.dma_start(out=pw["wwr"], in_=wwr[dj])
                        pw["wwi"] = wBp.tile([128, NFTP, 128], FP8, tag="wwi", name="pwwi")
                        nc.gpsimd.dma_start(out=pw["wwi"], in_=wwi[dj])
                        prefetchB[dj] = pw
                    sT2p = sTp.tile([128, 2, NM], FP8, tag="sT2")
                    nc.sync.dma_start(out=sT2p, in_=sT2w[0])
                    prefetchB["sT2_0"] = sT2p
                pend_mel = []
                for fi in range(NFT):
                    if fi == 0:
                        wc, ws = wc0, ws0
                    else:
                        wc = wpool.tile([128, NCH, 128], F32R, tag="wc")
                        nc.sync.dma_start(out=wc, in_=dftc[fi].bitcast(F32R))
                        ws = wpool.tile([128, NCH, 128], F32R, tag="ws")
                        nc.gpsimd.dma_start(out=ws, in_=dfts[fi].bitcast(F32R))
                    cL = psL.tile([128, hc], F32, tag="cL")
                    sL = psL.tile([128, hc], F32, tag="sL")
                    for c in range(NCH):
                        st, sp = (c == 0), (c == NCH - 1)
                        nc.tensor.matmul(cL, wc[:, c, :], fL[c][:, :hc],
                                         start=st, stop=sp, skip_group_check=True)
                        nc.tensor.matmul(sL, ws[:, c, :], fL[c][:, :hc],
                                         start=st, stop=sp, skip_group_check=True)
                    cR = psR.tile([128, hc], F32, tag="cR")
                    sR = psR.tile([128, hc], F32, tag="sR")
                    for c in range(NCH):
                        st, sp = (c == 0), (c == NCH - 1)
                        nc.tensor.matmul(cR, wc[:, c, :], fR[c][:, :hc],
                                         start=st, stop=sp, skip_group_check=True)
                        nc.tensor.matmul(sR, ws[:, c, :], fR[c][:, :hc],
                                         start=st, stop=sp, skip_group_check=True)
                    # emit the previous fi's mel matmuls now, so the PE never
                    # stalls on this fi's DVE/Act derivation chain
                    for mm_args in pend_mel:
                        nc.tensor.matmul(*mm_args[0], **mm_args[1])
                    pend_mel = []
                    if fi == NFM:
                        # fi-5's mel matmuls (the group stop) were just
                        # emitted: drain the mel accumulators now, well
                        # before the end-of-half scalar backlog
                        nc.scalar.copy(melri[:, hsl], pmc)
                        nc.scalar.copy(melLR[:, hsl], pmp)
                    # drain L PSUM early (psL bufs=1 gates the next fi)
                    aLr = tp2.tile([128, hc], F32, tag="aLr")
                    nc.scalar.copy(aLr, cL)
                    aLi = tp2.tile([128, hc], F32, tag="aLi")
                    nc.scalar.copy(aLi, sL)
                    # pow_L from SBUF copies; pow_R straight from PSUM
                    p1 = tp2.tile([128, hc], F32, tag="p1")
                    nc.scalar.activation(p1, aLr, AF.Square)
                    p2 = tp2.tile([128, hc], F32, tag="p2")
                    nc.scalar.activation(p2, aLi, AF.Square)
                    nc.vector.tensor_add(powL[:, fi, hsl], p1, p2)
                    p3 = tp2.tile([128, hc], F32, tag="p3")
                    nc.scalar.activation(p3, cR, AF.Square)
                    p4 = tp2.tile([128, hc], F32, tag="p4")
                    nc.scalar.activation(p4, sR, AF.Square)
                    nc.vector.tensor_add(powR[:, fi, hsl], p3, p4)
                    # csd = conj(X_L)*X_R ... (X_L * conj(X_R)): r = LrRr+LiRi,
                    # i = LiRr-LrRi
                    m1 = tp1.tile([128, hc], F32, tag="m1")
                    nc.vector.tensor_mul(m1, aLr, cR)
                    m2 = tp1.tile([128, hc], F32, tag="m2")
                    nc.vector.tensor_mul(m2, aLi, sR)
                    csdf = tp2.tile([128, hc], F32, tag="csdf")
                    nc.vector.tensor_add(csdf, m1, m2)
                    nc.scalar.copy(csdR[:, fi, hsl], csdf)
                    m3 = tp1.tile([128, hc], F32, tag="m3")
                    nc.vector.tensor_mul(m3, aLi, cR)
                    m4 = tp1.tile([128, hc], F32, tag="m4")
                    nc.vector.tensor_mul(m4, aLr, sR)
                    csif = tp2.tile([128, hc], F32, tag="csif")
                    nc.vector.tensor_sub(csif, m3, m4)
                    nc.scalar.copy(csdI[:, fi, hsl], csif)
                    if fi == 0:
                        pmc = psMel.tile([128, hc], F32, tag="pmc")
                        pmp = psMel.tile([128, hc], F32, tag="pmp")
                    if fi < NFM:
                        st, sp = (fi == 0), (fi == NFM - 1)
                        # csd mel in true fp32 (4 cyc/row): ch3/ch4 accuracy
                        pend_mel.append((
                            (pmc[0:NM, :], melwf_s[:, fi, :], csdf),
                            dict(start=st, stop=sp, skip_group_check=True)))
                        pend_mel.append((
                            (pmc[NM:128, :], melwf_s[:, fi, :], csif),
                            dict(start=st, stop=sp, skip_group_check=True)))
                        # pow mel in fp8
                        pend_mel.append((
                            (pmp[0:NM, :], melw_s[:, fi, :], powL[:, fi, hsl]),
                            dict(start=st, stop=sp, skip_group_check=True)))
                        pend_mel.append((
                            (pmp[NM:128, :], melw_s[:, fi, :], powR[:, fi, hsl]),
                            dict(start=st, stop=sp, skip_group_check=True)))
                for mm_args in pend_mel:
                    nc.tensor.matmul(*mm_args[0], **mm_args[1])
                if hx == 1:
                    for dj in (2, 3):
                        pw = {}
                        pw["whr"] = wBp.tile([128, NFTP, 128], FP8, tag="whr", name="pwhr2")
                        nc.sync.dma_start(out=pw["whr"], in_=whr[dj])
                        pw["whl"] = wBp.tile([128, NFTP, 128], FP8, tag="whl", name="pwhl2")
                        nc.gpsimd.dma_start(out=pw["whl"], in_=whl[dj])
                        pw["wwr"] = wBp.tile([128, NFTP, 128], FP8, tag="wwr", name="pwwr2")
                        nc.sync.dma_start(out=pw["wwr"], in_=wwr[dj])
                        pw["wwi"] = wBp.tile([128, NFTP, 128], FP8, tag="wwi", name="pwwi2")
                        nc.gpsimd.dma_start(out=pw["wwi"], in_=wwi[dj])
                        prefetchB[dj] = pw
                    sT2q = sTp.tile([128, 2, NM], FP8, tag="sT2")
                    nc.sync.dma_start(out=sT2q, in_=sT2w[1])
                    prefetchB["sT2_1"] = sT2q

        # ---------------- Phase C (ch0-ch4): finalize early ----------------
        # Only depends on phase A; output DMAs go on the vector queue so the
        # sync/gpsimd queues can start phase-B weight loads immediately.
        with tc.tile_pool(name="fint", bufs=1) as fin:
            # ch0/ch1 fused on the packed [128, TC] tile:
            # 10*log10(mel+eps) = K*ln(mel_s + eps*S_P) + C_LOG
            u01 = fin.tile([128, TC], F32, tag="u01")
            nc.scalar.activation(u01, melLR, AF.Ln, bias=ebM)
            c01 = fin.tile([128, TC], F32, tag="c01")
            nc.scalar.activation(c01, u01, AF.Identity, bias=cb0, scale=K_LOG)
            nc.scalar.dma_start(out=out[0], in_=c01[0:NM, :T])
            nc.scalar.dma_start(out=out[1], in_=c01[NM:128, :T])
            # ch2 = ch0 - ch1 up to O(eps/mel) terms; base-shift u1 to
            # partitions 0-63 first (DVE needs equal base partitions)
            u1c = fin.tile([NM, TC], F32, tag="u1c")
            nc.scalar.copy(u1c, u01[NM:128, :])
            w2 = fin.tile([NM, TC], F32, tag="w2")
            nc.vector.tensor_sub(w2, u01[0:NM, :], u1c)
            c2 = fin.tile([NM, TC], F32, tag="c2")
            nc.scalar.mul(c2, w2, K_LOG)
            nc.scalar.dma_start(out=out[2], in_=c2[:, :T])
            # ch3/ch4: rn = exp(-0.5*ln(r^2+i^2+eps))
            sq = fin.tile([128, TC], F32, tag="sq")
            nc.scalar.activation(sq, melri, AF.Square)
            sqi = fin.tile([NM, TC], F32, tag="sqi")
            nc.scalar.copy(sqi, sq[NM:128, :])
            ss = fin.tile([NM, TC], F32, tag="ss")
            nc.vector.tensor_add(ss, sq[0:NM, :], sqi)
            lnss = fin.tile([NM, TC], F32, tag="lnss")
            nc.scalar.activation(lnss, ss, AF.Ln, bias=ebN)
            rn = fin.tile([NM, TC], F32, tag="rn")
            nc.scalar.activation(rn, lnss, AF.Exp, scale=-0.5)
            mic = fin.tile([NM, TC], F32, tag="mic")
            nc.scalar.copy(mic, melri[NM:128, :])
            c3 = fin.tile([NM, TC], F32, tag="c3")
            nc.vector.tensor_mul(c3, mic, rn)
            nc.scalar.dma_start(out=out[3], in_=c3[:, :T])
            c4 = fin.tile([NM, TC], F32, tag="c4")
            nc.vector.tensor_mul(c4, melri[0:NM, :], rn)
            nc.scalar.dma_start(out=out[4], in_=c4[:, :T])

        # ---------------- Phase B: corr + scatter (fp8 DoubleRow) ----------
        # halves-inner ordering: consecutive matmul pairs share the same
        # lhsT slice, halving distinct weight loads if codegen elides them.
        with tc.tile_pool(name="pbt", bufs=2) as pbt, \
             tc.tile_pool(name="corrp", bufs=2) as corrp, \
             tc.tile_pool(name="psumB", bufs=1, space="PSUM") as psB, \
             tc.tile_pool(name="psumB5", bufs=1, space="PSUM") as psB5:
            ch5p = psB5.tile([NM, TC], F32, tag="ch5")
            pend_scatter = []
            c2 = None
            sT2 = None
            for dj in range(NDT):
                if dj in prefetchB:
                    pw = prefetchB.pop(dj)
                    whr_t, whl_t = pw["whr"], pw["whl"]
                    wwr_t, wwi_t = pw["wwr"], pw["wwi"]
                else:
                    whr_t = wBp.tile([128, NFTP, 128], FP8, tag="whr")
                    nc.sync.dma_start(out=whr_t, in_=whr[dj])
                    whl_t = wBp.tile([128, NFTP, 128], FP8, tag="whl")
                    nc.gpsimd.dma_start(out=whl_t, in_=whl[dj])
                    wwr_t = wBp.tile([128, NFTP, 128], FP8, tag="wwr")
                    nc.sync.dma_start(out=wwr_t, in_=wwr[dj])
                    wwi_t = wBp.tile([128, NFTP, 128], FP8, tag="wwi")
                    nc.gpsimd.dma_start(out=wwi_t, in_=wwi[dj])
                if dj % 2 == 0:
                    key = f"sT2_{dj // 2}"
                    if key in prefetchB:
                        sT2 = prefetchB.pop(key)
                    else:
                        sT2 = sTp.tile([128, 2, NM], FP8, tag="sT2")
                        nc.sync.dma_start(out=sT2, in_=sT2w[dj // 2])
                    c2 = [corrp.tile([128, 2, hc], FP8, tag=f"c2_{hh}",
                                     name=f"c2_{hh}")
                          for hh, (h0, hc) in enumerate(T_HALVES)]

                n1 = [psB.tile([128, hc], F32, tag=f"n1_{hh}", name=f"n1_{hh}")
                      for hh, (h0, hc) in enumerate(T_HALVES)]
                n2 = [psB.tile([128, hc], F32, tag=f"n2_{hh}", name=f"n2_{hh}")
                      for hh, (h0, hc) in enumerate(T_HALVES)]
                cu = [psB.tile([128, hc], F32, tag=f"cu_{hh}", name=f"cu_{hh}")
                      for hh, (h0, hc) in enumerate(T_HALVES)]
                # n1/n2 first, cu last: at the dj boundary the previous
                # dj's cu accumulator is freed by the END of its derivation
                # chain, so the next dj must not need cu's PSUM immediately
                for k in range(NFTP // 2):
                    ksl = slice(2 * k, 2 * k + 2)
                    st, sp = (k == 0), (k == NFTP // 2 - 1)
                    for hh, (h0, hc) in enumerate(T_HALVES):
                        nc.tensor.matmul(n1[hh], whr_t[:, ksl, :],
                                         powL[:, ksl, h0:h0 + hc],
                                         start=st, stop=sp,
                                         perf_mode=DROW, skip_group_check=True)
                    for hh, (h0, hc) in enumerate(T_HALVES):
                        nc.tensor.matmul(n2[hh], whl_t[:, ksl, :],
                                         powR[:, ksl, h0:h0 + hc],
                                         start=st, stop=sp,
                                         perf_mode=DROW, skip_group_check=True)
                for k in range(NFTP // 2):
                    ksl = slice(2 * k, 2 * k + 2)
                    st, sp = (k == 0), (k == NFTP // 2 - 1)
                    for hh, (h0, hc) in enumerate(T_HALVES):
                        nc.tensor.matmul(cu[hh], wwr_t[:, ksl, :],
                                         csdR[:, ksl, h0:h0 + hc],
                                         start=st, stop=False,
                                         perf_mode=DROW, skip_group_check=True)
                    for hh, (h0, hc) in enumerate(T_HALVES):
                        nc.tensor.matmul(cu[hh], wwi_t[:, ksl, :],
                                         csdI[:, ksl, h0:h0 + hc],
                                         start=False, stop=sp,
                                         perf_mode=DROW, skip_group_check=True)
                for hh, (h0, hc) in enumerate(T_HALVES):
                    cn1 = pbt.tile([128, hc], F32, tag="cn1")
                    nc.scalar.copy(cn1, n1[hh])
                    # drain cu to SBUF immediately: otherwise its PSUM bank
                    # is freed only by the corr mul at the END of the
                    # Ln/Exp chain, stalling the next dj's cu matmuls
                    ccu = pbt.tile([128, hc], F32, tag="ccu")
                    nc.scalar.copy(ccu, cu[hh])
                    den = pbt.tile([128, hc], F32, tag="den")
                    nc.vector.tensor_mul(den, cn1, n2[hh])
                    lnd = pbt.tile([128, hc], F32, tag="lnd")
                    nc.scalar.activation(lnd, den, AF.Ln, bias=ebB)
                    rden = pbt.tile([128, hc], F32, tag="rden")
                    nc.scalar.activation(rden, lnd, AF.Exp, scale=-0.5)
                    nc.vector.tensor_mul(c2[hh][:, dj % 2, :], ccu, rden)
                if dj % 2 == 1:
                    # defer the pair's scatter until after the NEXT pair's
                    # matmuls so the PE never stalls on the DVE chain
                    pend_scatter.append((sT2, c2, dj // 2))
                    if len(pend_scatter) > 1:
                        psT2, pc2, pp = pend_scatter.pop(0)
                        for hh, (h0, hc) in enumerate(T_HALVES):
                            nc.tensor.matmul(ch5p[:, h0:h0 + hc], psT2, pc2[hh],
                                             start=(pp == 0), stop=False,
                                             perf_mode=DROW,
                                             skip_group_check=True)
            for kk, (psT2, pc2, pp) in enumerate(pend_scatter):
                last = (kk == len(pend_scatter) - 1)
                for hh, (h0, hc) in enumerate(T_HALVES):
                    nc.tensor.matmul(ch5p[:, h0:h0 + hc], psT2, pc2[hh],
                                     start=(pp == 0), stop=last,
                                     perf_mode=DROW, skip_group_check=True)

            # ---------------- ch5 epilogue ----------------
            with tc.tile_pool(name="fin5", bufs=1) as fin5:
                c5 = fin5.tile([NM, TC], F32, tag="c5")
                nc.vector.tensor_scalar_mul(c5, ch5p, rcnt_s)
                nc.sync.dma_start(out=out[5], in_=c5[:, :T])

    fix_sync_waits(nc)
    return nc


def _host_prep(inputs):
    wav = np.asarray(inputs["waveform"], dtype=np.float32)          # [8,2,NS]
    W_real = np.asarray(inputs["W_real"], dtype=np.float32)         # [ND,F]
    W_imag = np.asarray(inputs["W_imag"], dtype=np.float32)
    norm_hr = np.asarray(inputs["norm_hr_sq"], dtype=np.float32)
    norm_hl = np.asarray(inputs["norm_hl_sq"], dtype=np.float32)
    az = np.asarray(inputs["az_bin_idx"]).astype(np.int64)          # [ND]
    win = np.asarray(inputs["window"], dtype=np.float32)            # [NFFT]
    mel_fb = np.asarray(inputs["mel_fb"], dtype=np.float32)         # [NM,F]

    xpad = np.pad(wav, ((0, 0), (0, 0), (NFFT // 2, NFFT // 2 + HOP)))  # [8,2,PADNS]
    # host framing: each (half, channel, chunk) SBUF tile is one fully
    # contiguous 256KB block in DRAM (single-descriptor DMAs)
    sw = np.lib.stride_tricks.sliding_window_view(xpad, NFFT, axis=2)[:, :, ::HOP]
    # sw: [8, 2, TC, NFFT] view -> [8, 2, NFFT, TC]
    frames_np = np.ascontiguousarray(np.swapaxes(sw, 2, 3))
    fr = np.zeros((B, 2, 2, NCH, 128, 512), dtype=np.float32)
    for hx, (t0, hc) in enumerate(T_HALVES):
        fr[:, :, hx, :, :, :hc] = frames_np[:, :, :, t0:t0 + hc].reshape(
            B, 2, NCH, 128, hc)

    n = np.arange(NFFT, dtype=np.float64)[:, None]
    k = np.arange(F, dtype=np.float64)[None, :]
    ang = 2.0 * np.pi * n * k / NFFT
    dftc = np.zeros((NFFT, FP), dtype=np.float32)
    dfts = np.zeros((NFFT, FP), dtype=np.float32)
    dftc[:, :F] = (np.cos(ang) * win[:, None] * S_X).astype(np.float32)
    dfts[:, :F] = (-np.sin(ang) * win[:, None] * S_X).astype(np.float32)

    def tile_dft(a):  # [NFFT, FP] -> [NFT, 128, NCH, 128]
        return np.ascontiguousarray(
            a.reshape(NCH, 128, NFT, 128).transpose(2, 1, 0, 3))

    def tile_w(mat):  # [ND, F] -> lhsT tiled [NDT, 128, NFTP, 128] fp8e4
        t = np.zeros((NFTP * 128, ND), dtype=np.float32)
        t[:F] = mat.T
        return np.ascontiguousarray(
            t.reshape(NFTP, 128, NDT, 128).transpose(2, 1, 0, 3)
        ).astype(ml_dtypes.float8_e4m3)

    wwr = tile_w(W_real)
    wwi = tile_w(-W_imag)
    whr = tile_w(norm_hr)
    whl = tile_w(norm_hl)

    # mel_fb is zero for bins >= 684, so dropping freq tiles 6-8 is exact
    melT = np.array(mel_fb.T[:NFM * 128], dtype=np.float32)  # [768, NM]
    mel_base = melT.reshape(NFM, 128, NM).transpose(1, 0, 2)  # [128, NFM, NM]
    melwf = np.ascontiguousarray(mel_base * np.float32(1.0 / S_P), dtype=np.float32)
    melw = np.ascontiguousarray(mel_base).astype(ml_dtypes.float8_e4m3)

    cnt = np.bincount(az, minlength=NM).astype(np.float32)
    S01 = (az[:, None] == np.arange(NM)[None, :]).astype(np.float32)  # [ND,NM]
    sT2w = np.ascontiguousarray(
        S01.reshape(NDT // 2, 2, 128, NM).transpose(0, 2, 1, 3)
    ).astype(ml_dtypes.float8_e4m3)
    rcnt = (1.0 / (cnt + EPS)).astype(np.float32).reshape(NM, 1)

    shared = {
        "dftc": tile_dft(dftc), "dfts": tile_dft(dfts),
        "wwr": wwr, "wwi": wwi, "whr": whr, "whl": whl,
        "melw": melw, "melwf": melwf, "sT2w": sT2w, "rcnt": rcnt,
    }
    in_maps = []
    for b in range(B):
        m = dict(shared)
        m["frames"] = fr[b]
        in_maps.append(m)
    return in_maps


def kernel(**inputs) -> np.ndarray:
    if "nc" not in _NC_CACHE:
        _NC_CACHE["nc"] = build_nc()
    nc = _NC_CACHE["nc"]
    in_maps = _host_prep(inputs)
    res = run_bass_kernel_spmd(nc, in_maps, core_ids=list(range(B)))
    out = np.stack([np.asarray(res.results[i]["out"]) for i in range(B)])
    return out.astype(np.float32)


# revision 34
# speedup vs baseline: 1.0169x; 1.0036x over previous
"""Trainium2 Bass kernel for nn_AudioPreprocessor (binaural STFT features).

Contract: kernel(**inputs) takes the FULL unsharded inputs (numpy) and
returns the full [8, 6, 64, 1001] float32 output. Internally: data-parallel
over batch across 8 NeuronCores (one batch per core, no collectives).

Pipeline per core (batch b):
  A) STFT of L/R channels as DFT-matmuls (f32r, weights pre-scaled by 1/16
     so pow/csd fit fp8e4 range), two time-halves (weights streamed twice,
     not four times); per (half, freq-tile): L channel accumulates in PSUM,
     drained to SBUF, then R channel; derive pow_L/R, csd_r/i into fp8e4
     residents (scaled 1/256, freq-padded 1025->1280 = 10 k-tiles for
     DoubleRow).  Mel matmuls run inline, deferred one freq-tile to keep
     the PE stream dense: csd path in true fp32 (ch3/ch4 are
     ill-conditioned), pow path in fp8.  No DRAM staging.
  B) per direction tile (18 x 128 dirs): n1/n2/corr_un as fp8 DoubleRow
     matmuls (2 k-tiles / 256 contraction per pass), normalize
     corr = corr_un * exp(-0.5*ln(n1*n2+eps)), scatter into azimuth bins
     via 0/1 matmul accumulated in PSUM.
  C) finalize ch0..ch4 right after phase A (overlaps phase B; all finalize
     work stays on the scalar queue so sync/gpsimd can start phase-B weight
     loads immediately); ch2 = ch0-ch1 algebraically; ch5 after the scatter.
"""
import sys
import os

_RL = "/opt/trn_rl_repo"
if _RL not in sys.path:
    sys.path.insert(0, _RL)

import numpy as np
import ml_dtypes
from contextlib import ExitStack

import concourse.bass as bass
import concourse.mybir as mybir
import concourse.tile as tile
from concourse.bass_utils import run_bass_kernel_spmd

# ---------------- problem constants (hardcoded) ----------------
B = 8
NS = 960000
NFFT = 2048
HOP = 960
F = 1025
FP = 1152            # padded freq dim: 9 * 128
NFT = FP // 128      # 9 freq tiles
NFTP = 10            # fp8 resid freq tiles (padded to even for DoubleRow)
NFM = 6              # freq tiles feeding mel (mel_fb zero above bin 683)
NCH = 16             # 2048 / 128 contraction chunks
NM = 64
ND = 2304
NDT = ND // 128      # 18 direction tiles
T = 1001
TC = 1002           # computed columns (f32r matmul needs even N); last col discarded
EPS = 1e-8
PADNS = NS + NFFT + HOP   # 963008 (one extra hop so we can compute T_C=1002 columns)
K_LOG = 10.0 / np.log(10.0)

# fp8 scaling: DFT weights pre-scaled by S_X -> X*S_X -> pow/csd * S_X^2.
S_X = 1.0 / 16.0
S_P = S_X * S_X                      # 1/256 scale on pow/csd
EPS_B = np.float32(EPS) * S_P * S_P  # phase-B rsqrt bias (n1*n2 scaled by S_P^2)
EPS_M = np.float32(EPS) * S_P        # mel ln bias (mel pow scaled by S_P)
C_LOG = 10.0 * np.log10(1.0 / S_P)   # ch0/ch1 offset compensating S_P

T_HALVES = [(0, 512), (512, 490)]    # matmul N<=512 splits of TC

F32 = mybir.dt.float32
F32R = mybir.dt.float32r
BF16 = mybir.dt.bfloat16
FP8 = mybir.dt.float8e4
AF = mybir.ActivationFunctionType
DROW = mybir.MatmulPerfMode.DoubleRow

_NC_CACHE = {}


def _budget(inst) -> int:
    return 1


def fix_sync_waits(nc):
    """Split per-instruction sync waits: this walrus build allows only ONE
    sync wait per instruction; hoist extras onto NoOps just before, on the
    same engine queue (engine streams execute in block order)."""
    counter = 0
    for f in nc.m.functions:
        for bb in f.blocks:
            new_insts = []
            changed = False
            for inst in bb.instructions:
                si = inst.sync_info
                waits = list(si.on_wait) if si is not None and si.on_wait else []
                budget = _budget(inst)
                if len(waits) > budget:
                    keep = waits[len(waits) - budget:]
                    excess = waits[: len(waits) - budget]
                    for wt in excess:
                        counter += 1
                        nop = mybir.InstNoOp(name=f"I-waitfix-{counter}")
                        nop.engine = inst.engine
                        nop.sync_info = mybir.SyncInfo(on_wait=[wt], on_update=[])
                        new_insts.append(nop)
                    inst.sync_info = mybir.SyncInfo(
                        on_wait=keep,
                        on_update=list(si.on_update) if si.on_update else [],
                    )
                    changed = True
                new_insts.append(inst)
            if changed:
                bb.instructions = new_insts
    return nc


def build_nc():
    nc = bass.Bass()

    frames = nc.declare_dram_parameter("frames", [2, 2, NCH, 128, 512], F32, isOutput=False)
    dftc = nc.declare_dram_parameter("dftc", [NFT, 128, NCH, 128], F32, isOutput=False)
    dfts = nc.declare_dram_parameter("dfts", [NFT, 128, NCH, 128], F32, isOutput=False)
    wwr = nc.declare_dram_parameter("wwr", [NDT, 128, NFTP, 128], FP8, isOutput=False)
    wwi = nc.declare_dram_parameter("wwi", [NDT, 128, NFTP, 128], FP8, isOutput=False)
    whr = nc.declare_dram_parameter("whr", [NDT, 128, NFTP, 128], FP8, isOutput=False)
    whl = nc.declare_dram_parameter("whl", [NDT, 128, NFTP, 128], FP8, isOutput=False)
    melw = nc.declare_dram_parameter("melw", [128, NFM, NM], FP8, isOutput=False)
    melwf = nc.declare_dram_parameter("melwf", [128, NFM, NM], F32, isOutput=False)
    sT2w = nc.declare_dram_parameter("sT2w", [NDT // 2, 128, 2, NM], FP8, isOutput=False)
    rcnt = nc.declare_dram_parameter("rcnt", [NM, 1], F32, isOutput=False)
    out = nc.declare_dram_parameter("out", [6, NM, T], F32, isOutput=True)

    with tile.TileContext(nc) as tc, ExitStack() as ctx:
        # resident pow/csd arrays [128, 10, 1002] fp8 (scaled by 1/256)
        resid = ctx.enter_context(tc.tile_pool(name="resid", bufs=1))
        powL = resid.tile([128, NFTP, TC], FP8, tag="powL")
        powR = resid.tile([128, NFTP, TC], FP8, tag="powR")
        csdR = resid.tile([128, NFTP, TC], FP8, tag="csdR")
        csdI = resid.tile([128, NFTP, TC], FP8, tag="csdI")
        # zero the padded k-tile (9): DoubleRow reads it; weights there are 0,
        # but uninitialized fp8 could be inf/nan and 0*inf = nan in the PE.
        for rt_ in (powL, powR, csdR, csdI):
            nc.vector.memset(rt_[:, 9, :], 0.0)
        # mel/channel tiles [64, 1002] f32 + consts
        mels = ctx.enter_context(tc.tile_pool(name="mels", bufs=1))
        melLR = mels.tile([128, TC], F32, tag="melLR")
        melri = mels.tile([128, TC], F32, tag="melri")
        melL_s, melR_s = melLR[0:NM, :], melLR[NM:128, :]
        melr_s, meli_s = melri[0:NM, :], melri[NM:128, :]
        ebB = mels.tile([128, 1], F32, tag="ebB")      # phase-B bias eps*S_P^2
        ebM = mels.tile([128, 1], F32, tag="ebM")      # mel ln bias eps*S_P
        ebN = mels.tile([NM, 1], F32, tag="ebN")       # ch3/4 norm bias eps
        cb0 = mels.tile([128, 1], F32, tag="cb0")      # ch0/1 +10*log10(256)
        rcnt_s = mels.tile([NM, 1], F32, tag="rcnt")
        nc.vector.memset(ebB, float(EPS_B))
        nc.vector.memset(ebM, float(EPS_M))
        nc.vector.memset(ebN, float(EPS))
        nc.vector.memset(cb0, float(C_LOG))
        # mel weights resident through phase A (tiles allocated here; their
        # DMAs are emitted after the half-0 frame loads so the prologue's
        # critical wc0+fL0 transfers aren't queued behind them)
        melwp = ctx.enter_context(tc.tile_pool(name="melwp", bufs=1))
        melw_s = melwp.tile([128, NFM, NM], FP8, tag="melw")
        melwf_s = melwp.tile([128, NFM, NM], F32, tag="melwf")
        # phase-B weight pools live for the whole kernel so the first two
        # direction tiles can prefetch during phase A's second half
        wBp = ctx.enter_context(tc.tile_pool(name="wB", bufs=4))
        sTp = ctx.enter_context(tc.tile_pool(name="sTp", bufs=2))
        prefetchB = {}

        # ---------------- Phase A: STFT + pow/csd + inline mel -------------
        with tc.tile_pool(name="frames", bufs=1) as fpool, \
             tc.tile_pool(name="dftw", bufs=2) as wpool, \
             tc.tile_pool(name="pa2", bufs=2) as tp2, \
             tc.tile_pool(name="pa1", bufs=1) as tp1, \
             tc.tile_pool(name="psL", bufs=1, space="PSUM") as psL, \
             tc.tile_pool(name="psR", bufs=2, space="PSUM") as psR, \
             tc.tile_pool(name="psmel", bufs=1, space="PSUM") as psMel:
            for hx, (t0, hc) in enumerate(T_HALVES):
                hsl = slice(t0, t0 + hc)
                fL, fR = {}, {}
                wcA = wcB = None
                if hx == 0:
                    # fi-0 cos weights split in half so the very first
                    # matmul waits on 0.5MB + one frame chunk, not 1.3MB
                    # both wc ring buffers, half-filled: no extra SBUF
                    wcA = wpool.tile([128, NCH, 128], F32R, tag="wc")
                    nc.sync.dma_start(out=wcA[:, :NCH // 2, :],
                                      in_=dftc[0, :, :NCH // 2, :].bitcast(F32R))
                    ws0 = wpool.tile([128, NCH, 128], F32R, tag="ws")
                    nc.gpsimd.dma_start(out=ws0, in_=dfts[0].bitcast(F32R))
                for c in range(NCH):
                    ft = fpool.tile([128, 512], F32R, tag=f"fL{c}")
                    eng = nc.sync if (c % 2 == 0) else nc.gpsimd
                    eng.dma_start(out=ft, in_=frames[0, hx, c].bitcast(F32R))
                    fL[c] = ft
                    if hx == 0 and c == 0:
                        wcB = wpool.tile([128, NCH, 128], F32R, tag="wc")
                        nc.sync.dma_start(
                            out=wcB[:, :NCH // 2, :],
                            in_=dftc[0, :, NCH // 2:, :].bitcast(F32R))
                if hx == 1:
                    wc0 = wpool.tile([128, NCH, 128], F32R, tag="wc")
                    nc.sync.dma_start(out=wc0, in_=dftc[0].bitcast(F32R))
                    ws0 = wpool.tile([128, NCH, 128], F32R, tag="ws")
                    nc.gpsimd.dma_start(out=ws0, in_=dfts[0].bitcast(F32R))
                for c in range(NCH):
                    ft = fpool.tile([128, 512], F32R, tag=f"fR{c}")
                    eng = nc.sync if (c % 2 == 0) else nc.gpsimd
                    eng.dma_start(out=ft, in_=frames[1, hx, c].bitcast(F32R))
                    fR[c] = ft
                if hx == 0:
                    nc.sync.dma_start(out=rcnt_s, in_=rcnt[:, :])
                    nc.sync.dma_start(out=melw_s, in_=melw[:, :, :])
                    nc.gpsimd.dma_start(out=melwf_s, in_=melwf[:, :, :])
                if hx == 1:
                    # prefetch the first two phase-B direction tiles' weights
                    # during the second STFT half, so phase B starts hot
                    for dj in (0, 1):
                        pw = {}
                        pw["whr"] = wBp.tile([128, NFTP, 128], FP8, tag="whr", name="pwhr")
                        nc.sync.dma_start(out=pw["whr"], in_=whr[dj])
                        pw["whl"] = wBp.tile([128, NFTP, 128], FP8, tag="whl", name="pwhl")
                        nc.gpsimd.dma_start(out=pw["whl"], in_=whl[dj])
                        pw["wwr"] = wBp.tile([128, NFTP, 128], FP8, tag="wwr", name="pwwr")
                        nc.sync.dma_start(out=pw["wwr"], in_=wwr[dj])
                        pw["wwi"] = wBp.tile([128, NFTP, 128], FP8, tag="wwi", name="pwwi")
                        nc.gpsimd.dma_start(out=pw["wwi"], in_=wwi[dj])
                        prefetchB[dj] = pw
                    sT2p = sTp.tile([128, 2, NM], FP8, tag="sT2")
                    nc.sync.dma_start(out=sT2p, in_=sT2w[0])
                    prefetchB["sT2_0"] = sT2p
                pend_mel = []
                for fi in range(NFT):
                    if fi == 0:
                        wc = None if hx == 0 else wc0
                        ws = ws0
                    else:
                        wc = wpool.tile([128, NCH, 128], F32R, tag="wc")
                        nc.sync.dma_start(out=wc, in_=dftc[fi].bitcast(F32R))
                        ws = wpool.tile([128, NCH, 128], F32R, tag="ws")
                        nc.gpsimd.dma_start(out=ws, in_=dfts[fi].bitcast(F32R))
                    cL = psL.tile([128, hc], F32, tag="cL")
                    sL = psL.tile([128, hc], F32, tag="sL")
                    for c in range(NCH):
                        st, sp = (c == 0), (c == NCH - 1)
                        wcs = (wc[:, c, :] if wc is not None else
                               (wcA[:, c, :] if c < NCH // 2 else
                                wcB[:, c - NCH // 2, :]))
                        nc.tensor.matmul(cL, wcs, fL[c][:, :hc],
                                         start=st, stop=sp, skip_group_check=True)
                        nc.tensor.matmul(sL, ws[:, c, :], fL[c][:, :hc],
                                         start=st, stop=sp, skip_group_check=True)
                    cR = psR.tile([128, hc], F32, tag="cR")
                    sR = psR.tile([128, hc], F32, tag="sR")
                    for c in range(NCH):
                        st, sp = (c == 0), (c == NCH - 1)
                        wcs = (wc[:, c, :] if wc is not None else
                               (wcA[:, c, :] if c < NCH // 2 else
                                wcB[:, c - NCH // 2, :]))
                        nc.tensor.matmul(cR, wcs, fR[c][:, :hc],
                                         start=st, stop=sp, skip_group_check=True)
                        nc.tensor.matmul(sR, ws[:, c, :], fR[c][:, :hc],
                                         start=st, stop=sp, skip_group_check=True)
                    # emit the previous fi's mel matmuls now, so the PE never
                    # stalls on this fi's DVE/Act derivation chain
                    for mm_args in pend_mel:
                        nc.tensor.matmul(*mm_args[0], **mm_args[1])
                    pend_mel = []
                    if fi == NFM:
                        # fi-5's mel matmuls (the group stop) were just
                        # emitted: drain the mel accumulators now, well
                        # before the end-of-half scalar backlog
                        nc.scalar.copy(melri[:, hsl], pmc)
                        nc.scalar.copy(melLR[:, hsl], pmp)
                    # drain L PSUM early (psL bufs=1 gates the next fi)
                    aLr = tp2.tile([128, hc], F32, tag="aLr")
                    nc.scalar.copy(aLr, cL)
                    aLi = tp2.tile([128, hc], F32, tag="aLi")
                    nc.scalar.copy(aLi, sL)
                    # pow_L from SBUF copies; pow_R straight from PSUM
                    p1 = tp2.tile([128, hc], F32, tag="p1")
                    nc.scalar.activation(p1, aLr, AF.Square)
                    p2 = tp2.tile([128, hc], F32, tag="p2")
                    nc.scalar.activation(p2, aLi, AF.Square)
                    nc.vector.tensor_add(powL[:, fi, hsl], p1, p2)
                    p3 = tp2.tile([128, hc], F32, tag="p3")
                    nc.scalar.activation(p3, cR, AF.Square)
                    p4 = tp2.tile([128, hc], F32, tag="p4")
                    nc.scalar.activation(p4, sR, AF.Square)
                    nc.vector.tensor_add(powR[:, fi, hsl], p3, p4)
                    # csd = conj(X_L)*X_R ... (X_L * conj(X_R)): r = LrRr+LiRi,
                    # i = LiRr-LrRi
                    m1 = tp1.tile([128, hc], F32, tag="m1")
                    nc.vector.tensor_mul(m1, aLr, cR)
                    m2 = tp1.tile([128, hc], F32, tag="m2")
                    nc.vector.tensor_mul(m2, aLi, sR)
                    csdf = tp2.tile([128, hc], F32, tag="csdf")
                    nc.vector.tensor_add(csdf, m1, m2)
                    nc.scalar.copy(csdR[:, fi, hsl], csdf)
                    m3 = tp1.tile([128, hc], F32, tag="m3")
                    nc.vector.tensor_mul(m3, aLi, cR)
                    m4 = tp1.tile([128, hc], F32, tag="m4")
                    nc.vector.tensor_mul(m4, aLr, sR)
                    csif = tp2.tile([128, hc], F32, tag="csif")
                    nc.vector.tensor_sub(csif, m3, m4)
                    nc.scalar.copy(csdI[:, fi, hsl], csif)
                    if fi == 0:
                        pmc = psMel.tile([128, hc], F32, tag="pmc")
                        pmp = psMel.tile([128, hc], F32, tag="pmp")
                    if fi < NFM:
                        st, sp = (fi == 0), (fi == NFM - 1)
                        # csd mel in true fp32 (4 cyc/row): ch3/ch4 accuracy
                        pend_mel.append((
                            (pmc[0:NM, :], melwf_s[:, fi, :], csdf),
                            dict(start=st, stop=sp, skip_group_check=True)))
                        pend_mel.append((
                            (pmc[NM:128, :], melwf_s[:, fi, :], csif),
                            dict(start=st, stop=sp, skip_group_check=True)))
                        # pow mel in fp8
                        pend_mel.append((
                            (pmp[0:NM, :], melw_s[:, fi, :], powL[:, fi, hsl]),
                            dict(start=st, stop=sp, skip_group_check=True)))
                        pend_mel.append((
                            (pmp[NM:128, :], melw_s[:, fi, :], powR[:, fi, hsl]),
                            dict(start=st, stop=sp, skip_group_check=True)))
                for mm_args in pend_mel:
                    nc.tensor.matmul(*mm_args[0], **mm_args[1])
                if hx == 1:
                    for dj in (2, 3):
                        pw = {}
                        pw["whr"] = wBp.tile([128, NFTP, 128], FP8, tag="whr", name="pwhr2")
                        nc.sync.dma_start(out=pw["whr"], in_=whr[dj])
                        pw["whl"] = wBp.tile([128, NFTP, 128], FP8, tag="whl", name="pwhl2")
                        nc.gpsimd.dma_start(out=pw["whl"], in_=whl[dj])
                        pw["wwr"] = wBp.tile([128, NFTP, 128], FP8, tag="wwr", name="pwwr2")
                        nc.sync.dma_start(out=pw["wwr"], in_=wwr[dj])
                        pw["wwi"] = wBp.tile([128, NFTP, 128], FP8, tag="wwi", name="pwwi2")
                        nc.gpsimd.dma_start(out=pw["wwi"], in_=wwi[dj])
                        prefetchB[dj] = pw
                    sT2q = sTp.tile([128, 2, NM], FP8, tag="sT2")
                    nc.sync.dma_start(out=sT2q, in_=sT2w[1])
                    prefetchB["sT2_1"] = sT2q

        # ---------------- Phase C (ch0-ch4): finalize early ----------------
        # Only depends on phase A; output DMAs go on the vector queue so the
        # sync/gpsimd queues can start phase-B weight loads immediately.
        with tc.tile_pool(name="fint", bufs=1) as fin:
            # ch0/ch1 fused on the packed [128, TC] tile:
            # 10*log10(mel+eps) = K*ln(mel_s + eps*S_P) + C_LOG
            u01 = fin.tile([128, TC], F32, tag="u01")
            nc.scalar.activation(u01, melLR, AF.Ln, bias=ebM)
            c01 = fin.tile([128, TC], F32, tag="c01")
            nc.scalar.activation(c01, u01, AF.Identity, bias=cb0, scale=K_LOG)
            nc.scalar.dma_start(out=out[0], in_=c01[0:NM, :T])
            nc.scalar.dma_start(out=out[1], in_=c01[NM:128, :T])
            # ch2 = ch0 - ch1 up to O(eps/mel) terms; base-shift u1 to
            # partitions 0-63 first (DVE needs equal base partitions)
            u1c = fin.tile([NM, TC], F32, tag="u1c")
            nc.scalar.copy(u1c, u01[NM:128, :])
            w2 = fin.tile([NM, TC], F32, tag="w2")
            nc.vector.tensor_sub(w2, u01[0:NM, :], u1c)
            c2 = fin.tile([NM, TC], F32, tag="c2")
            nc.scalar.mul(c2, w2, K_LOG)
            nc.scalar.dma_start(out=out[2], in_=c2[:, :T])
            # ch3/ch4: rn = exp(-0.5*ln(r^2+i^2+eps))
            sq = fin.tile([128, TC], F32, tag="sq")
            nc.scalar.activation(sq, melri, AF.Square)
            sqi = fin.tile([NM, TC], F32, tag="sqi")
            nc.scalar.copy(sqi, sq[NM:128, :])
            ss = fin.tile([NM, TC], F32, tag="ss")
            nc.vector.tensor_add(ss, sq[0:NM, :], sqi)
            lnss = fin.tile([NM, TC], F32, tag="lnss")
            nc.scalar.activation(lnss, ss, AF.Ln, bias=ebN)
            rn = fin.tile([NM, TC], F32, tag="rn")
            nc.scalar.activation(rn, lnss, AF.Exp, scale=-0.5)
            mic = fin.tile([NM, TC], F32, tag="mic")
            nc.scalar.copy(mic, melri[NM:128, :])
            c3 = fin.tile([NM, TC], F32, tag="c3")
            nc.vector.tensor_mul(c3, mic, rn)
            nc.scalar.dma_start(out=out[3], in_=c3[:, :T])
            c4 = fin.tile([NM, TC], F32, tag="c4")
            nc.vector.tensor_mul(c4, melri[0:NM, :], rn)
            nc.scalar.dma_start(out=out[4], in_=c4[:, :T])

        # ---------------- Phase B: corr + scatter (fp8 DoubleRow) ----------
        # halves-inner ordering: consecutive matmul pairs share the same
        # lhsT slice, halving distinct weight loads if codegen elides them.
        with tc.tile_pool(name="pbt", bufs=2) as pbt, \
             tc.tile_pool(name="corrp", bufs=2) as corrp, \
             tc.tile_pool(name="psumB", bufs=1, space="PSUM") as psB, \
             tc.tile_pool(name="psumB5", bufs=1, space="PSUM") as psB5:
            ch5p = psB5.tile([NM, TC], F32, tag="ch5")
            pend_scatter = []
            c2 = None
            sT2 = None
            for dj in range(NDT):
                if dj in prefetchB:
                    pw = prefetchB.pop(dj)
                    whr_t, whl_t = pw["whr"], pw["whl"]
                    wwr_t, wwi_t = pw["wwr"], pw["wwi"]
                else:
                    whr_t = wBp.tile([128, NFTP, 128], FP8, tag="whr")
                    nc.sync.dma_start(out=whr_t, in_=whr[dj])
                    whl_t = wBp.tile([128, NFTP, 128], FP8, tag="whl")
                    nc.gpsimd.dma_start(out=whl_t, in_=whl[dj])
                    wwr_t = wBp.tile([128, NFTP, 128], FP8, tag="wwr")
                    nc.sync.dma_start(out=wwr_t, in_=wwr[dj])
                    wwi_t = wBp.tile([128, NFTP, 128], FP8, tag="wwi")
                    nc.gpsimd.dma_start(out=wwi_t, in_=wwi[dj])
                if dj % 2 == 0:
                    key = f"sT2_{dj // 2}"
                    if key in prefetchB:
                        sT2 = prefetchB.pop(key)
                    else:
                        sT2 = sTp.tile([128, 2, NM], FP8, tag="sT2")
                        nc.sync.dma_start(out=sT2, in_=sT2w[dj // 2])
                    c2 = [corrp.tile([128, 2, hc], FP8, tag=f"c2_{hh}",
                                     name=f"c2_{hh}")
                          for hh, (h0, hc) in enumerate(T_HALVES)]

                n1 = [psB.tile([128, hc], F32, tag=f"n1_{hh}", name=f"n1_{hh}")
                      for hh, (h0, hc) in enumerate(T_HALVES)]
                n2 = [psB.tile([128, hc], F32, tag=f"n2_{hh}", name=f"n2_{hh}")
                      for hh, (h0, hc) in enumerate(T_HALVES)]
                cu = [psB.tile([128, hc], F32, tag=f"cu_{hh}", name=f"cu_{hh}")
                      for hh, (h0, hc) in enumerate(T_HALVES)]
                # n1/n2 first, cu last: at the dj boundary the previous
                # dj's cu accumulator is freed by the END of its derivation
                # chain, so the next dj must not need cu's PSUM immediately
                for k in range(NFTP // 2):
                    ksl = slice(2 * k, 2 * k + 2)
                    st, sp = (k == 0), (k == NFTP // 2 - 1)
                    for hh, (h0, hc) in enumerate(T_HALVES):
                        nc.tensor.matmul(n1[hh], whr_t[:, ksl, :],
                                         powL[:, ksl, h0:h0 + hc],
                                         start=st, stop=sp,
                                         perf_mode=DROW, skip_group_check=True)
                    for hh, (h0, hc) in enumerate(T_HALVES):
                        nc.tensor.matmul(n2[hh], whl_t[:, ksl, :],
                                         powR[:, ksl, h0:h0 + hc],
                                         start=st, stop=sp,
                                         perf_mode=DROW, skip_group_check=True)
                for k in range(NFTP // 2):
                    ksl = slice(2 * k, 2 * k + 2)
                    st, sp = (k == 0), (k == NFTP // 2 - 1)
                    for hh, (h0, hc) in enumerate(T_HALVES):
                        nc.tensor.matmul(cu[hh], wwr_t[:, ksl, :],
                                         csdR[:, ksl, h0:h0 + hc],
                                         start=st, stop=False,
                                         perf_mode=DROW, skip_group_check=True)
                    for hh, (h0, hc) in enumerate(T_HALVES):
                        nc.tensor.matmul(cu[hh], wwi_t[:, ksl, :],
                                         csdI[:, ksl, h0:h0 + hc],
                                         start=False, stop=sp,
                                         perf_mode=DROW, skip_group_check=True)
                for hh, (h0, hc) in enumerate(T_HALVES):
                    cn1 = pbt.tile([128, hc], F32, tag="cn1")
                    nc.scalar.copy(cn1, n1[hh])
                    den = pbt.tile([128, hc], F32, tag="den")
                    nc.vector.tensor_mul(den, cn1, n2[hh])
                    lnd = pbt.tile([128, hc], F32, tag="lnd")
                    nc.scalar.activation(lnd, den, AF.Ln, bias=ebB)
                    rden = pbt.tile([128, hc], F32, tag="rden")
                    nc.scalar.activation(rden, lnd, AF.Exp, scale=-0.5)
                    nc.vector.tensor_mul(c2[hh][:, dj % 2, :], cu[hh], rden)
                if dj % 2 == 1:
                    # defer the pair's scatter until after the NEXT pair's
                    # matmuls so the PE never stalls on the DVE chain
                    pend_scatter.append((sT2, c2, dj // 2))
                    if len(pend_scatter) > 1:
                        psT2, pc2, pp = pend_scatter.pop(0)
                        for hh, (h0, hc) in enumerate(T_HALVES):
                            nc.tensor.matmul(ch5p[:, h0:h0 + hc], psT2, pc2[hh],
                                             start=(pp == 0), stop=False,
                                             perf_mode=DROW,
                                             skip_group_check=True)
            for kk, (psT2, pc2, pp) in enumerate(pend_scatter):
                last = (kk == len(pend_scatter) - 1)
                for hh, (h0, hc) in enumerate(T_HALVES):
                    nc.tensor.matmul(ch5p[:, h0:h0 + hc], psT2, pc2[hh],
                                     start=(pp == 0), stop=last,
                                     perf_mode=DROW, skip_group_check=True)

            # ---------------- ch5 epilogue ----------------
            with tc.tile_pool(name="fin5", bufs=1) as fin5:
                c5 = fin5.tile([NM, TC], F32, tag="c5")
                nc.vector.tensor_scalar_mul(c5, ch5p, rcnt_s)
                nc.sync.dma_start(out=out[5], in_=c5[:, :T])

    fix_sync_waits(nc)
    return nc


def _host_prep(inputs):
    wav = np.asarray(inputs["waveform"], dtype=np.float32)          # [8,2,NS]
    W_real = np.asarray(inputs["W_real"], dtype=np.float32)         # [ND,F]
    W_imag = np.asarray(inputs["W_imag"], dtype=np.float32)
    norm_hr = np.asarray(inputs["norm_hr_sq"], dtype=np.float32)
    norm_hl = np.asarray(inputs["norm_hl_sq"], dtype=np.float32)
    az = np.asarray(inputs["az_bin_idx"]).astype(np.int64)          # [ND]
    win = np.asarray(inputs["window"], dtype=np.float32)            # [NFFT]
    mel_fb = np.asarray(inputs["mel_fb"], dtype=np.float32)         # [NM,F]

    xpad = np.pad(wav, ((0, 0), (0, 0), (NFFT // 2, NFFT // 2 + HOP)))  # [8,2,PADNS]
    # host framing: each (half, channel, chunk) SBUF tile is one fully
    # contiguous 256KB block in DRAM (single-descriptor DMAs)
    sw = np.lib.stride_tricks.sliding_window_view(xpad, NFFT, axis=2)[:, :, ::HOP]
    # sw: [8, 2, TC, NFFT] view -> [8, 2, NFFT, TC]
    frames_np = np.ascontiguousarray(np.swapaxes(sw, 2, 3))
    fr = np.zeros((B, 2, 2, NCH, 128, 512), dtype=np.float32)
    for hx, (t0, hc) in enumerate(T_HALVES):
        fr[:, :, hx, :, :, :hc] = frames_np[:, :, :, t0:t0 + hc].reshape(
            B, 2, NCH, 128, hc)

    n = np.arange(NFFT, dtype=np.float64)[:, None]
    k = np.arange(F, dtype=np.float64)[None, :]
    ang = 2.0 * np.pi * n * k / NFFT
    dftc = np.zeros((NFFT, FP), dtype=np.float32)
    dfts = np.zeros((NFFT, FP), dtype=np.float32)
    dftc[:, :F] = (np.cos(ang) * win[:, None] * S_X).astype(np.float32)
    dfts[:, :F] = (-np.sin(ang) * win[:, None] * S_X).astype(np.float32)

    def tile_dft(a):  # [NFFT, FP] -> [NFT, 128, NCH, 128]
        return np.ascontiguousarray(
            a.reshape(NCH, 128, NFT, 128).transpose(2, 1, 0, 3))

    def tile_w(mat):  # [ND, F] -> lhsT tiled [NDT, 128, NFTP, 128] fp8e4
        t = np.zeros((NFTP * 128, ND), dtype=np.float32)
        t[:F] = mat.T
        return np.ascontiguousarray(
            t.reshape(NFTP, 128, NDT, 128).transpose(2, 1, 0, 3)
        ).astype(ml_dtypes.float8_e4m3)

    wwr = tile_w(W_real)
    wwi = tile_w(-W_imag)
    whr = tile_w(norm_hr)
    whl = tile_w(norm_hl)

    # mel_fb is zero for bins >= 684, so dropping freq tiles 6-8 is exact
    melT = np.array(mel_fb.T[:NFM * 128], dtype=np.float32)  # [768, NM]
    mel_base = melT.reshape(NFM, 128, NM).transpose(1, 0, 2)  # [128, NFM, NM]
    melwf = np.ascontiguousarray(mel_base * np.float32(1.0 / S_P), dtype=np.float32)
    melw = np.ascontiguousarray(mel_base).astype(ml_dtypes.float8_e4m3)

    cnt = np.bincount(az, minlength=NM).astype(np.float32)
    S01 = (az[:, None] == np.arange(NM)[None, :]).astype(np.float32)  # [ND,NM]
    sT2w = np.ascontiguousarray(
        S01.reshape(NDT // 2, 2, 128, NM).transpose(0, 2, 1, 3)
    ).astype(ml_dtypes.float8_e4m3)
    rcnt = (1.0 / (cnt + EPS)).astype(np.float32).reshape(NM, 1)

    shared = {
        "dftc": tile_dft(dftc), "dfts": tile_dft(dfts),
        "wwr": wwr, "wwi": wwi, "whr": whr, "whl": whl,
        "melw": melw, "melwf": melwf, "sT2w": sT2w, "rcnt": rcnt,
    }
    in_maps = []
    for b in range(B):
        m = dict(shared)
        m["frames"] = fr[b]
        in_maps.append(m)
    return in_maps


def kernel(**inputs) -> np.ndarray:
    if "nc" not in _NC_CACHE:
        _NC_CACHE["nc"] = build_nc()
    nc = _NC_CACHE["nc"]
    in_maps = _host_prep(inputs)
    res = run_bass_kernel_spmd(nc, in_maps, core_ids=list(range(B)))
    out = np.stack([np.asarray(res.results[i]["out"]) for i in range(B)])
    return out.astype(np.float32)
